# revision 20
# baseline (speedup 1.0000x reference)
"""Trainium2 Bass kernel for gated 1x1-conv attention (dense_transformer).

Problem structure (B=4, C=3, H=W=64, heads=3 => c_h=1): attention logits are
rank-1: att[n] = softmax_m(q_n * k_m) @ v over N=4096 pixels; a luma gate
scales q; the 1x1 convs are 3x3 channel mixes.

Sharding: 8 cores = (batch b = j//2) x (query-pixel half = j%2); each core
produces the full RGB output for its 2048 query pixels. No collectives.

Algorithm (Gaussian-quadrature factorization of the exp kernel): per head,
over a T=32 grid t_j with spacing hg and sigma = hg,
  e^{q k} = e^{-s^2 k^2/2} (hg/(s sqrt(2pi))) sum_j e^{-(q-t_j)^2/(2s^2)} e^{t_j k}
This collapses the N x N attention to N x T + T x N work:
  grid:  gnum[j] = sum_m v_m e^{logit(j,m)},  gden[j] = sum_m e^{logit(j,m)}
         with logit = t_j k_m - s^2 k_m^2 / 2 (the k^2 correction kept exactly)
  W:     W[j, n] = e^{t_j q_n / s^2 - t_j^2/(2 s^2)};  att = (W.T gnum)/(W.T gden)
(the per-column e^{-q^2/2s^2} factor cancels in the ratio; measured exp args
stay < +16 so no overflow without it).

v2 restructure vs the 86 us baseline: luma statistics moved to a [128, 32]
column layout (vector-engine ops + two tiny partition-reduce matmuls) instead
of row-broadcast matmuls + ScalarE accumulation; the gate uses only
Abs/Tanh/Exp so a single activation table load suffices (no Ln -> no table
swap); the W exponent is built by a K=3 f32r matmul with the per-row
-t^2/2s^2 folded into the Exp bias (kills the Square pass and the broadcast
matmul); all host-prepared constants arrive in one f32 blob + one bf16 blob
(fewer DMA round trips); no PE warm-up loop.
"""

import numpy as np

import concourse.bass as bass
import concourse.bacc as bacc
import concourse.mybir as mybir
from concourse.tile import TileContext
from concourse.bass_utils import run_bass_kernel_spmd

F32 = mybir.dt.float32
F32R = mybir.dt.float32r
BF16 = mybir.dt.bfloat16
AF = mybir.ActivationFunctionType
ALU = mybir.AluOpType
AX = mybir.AxisListType

N = 4096          # pixels per image
NSL = 2048        # query pixels per core
NQT = 16          # query tiles of 128
P = 128
T = 32            # quadrature grid points per head (3 blocks of 32)
LUMW = (0.299, 0.587, 0.114)
NCH = 4           # key chunks of 1024 in the grid build
CH = N // NCH

# f32 blob column map
FB_IMG = 0        # [128, 3, 32] imgcol (c-major: col = c*32 + ct)
FB_TEXP = 96      # [128, 1] -t^2/(2 s^2) per grid row (pad rows -100)
FB_WO = 97        # [128, 9] wo replicated, col 97+3c+h = wo[c, h]
FB_WSTK = 106     # rows 0:3, cols 106:234: wstk3 [3, 128] t_j/s^2 blocks
FB_WQT = 234      # rows 0:3, cols 234:237: wq^T
FB_LUM = 237      # rows 0:3, cols 237:240: lum coef replicated (lumrep)
FB_W = 240


def build_nc(debug=False):
    nc = bacc.Bacc("TRN2", target_bir_lowering=False, debug=False,
                   num_devices=8)

    fblob = nc.declare_dram_parameter("fblob", [P, FB_W], F32, isOutput=False)
    imgstack = nc.declare_dram_parameter("imgstack", [15, N], BF16,
                                         isOutput=False)
    imghi3 = nc.declare_dram_parameter("imghi3", [3, N], BF16, isOutput=False)
    qimg = nc.declare_dram_parameter("qimg", [3, NSL], F32R, isOutput=False)
    bfblob = nc.declare_dram_parameter("bfblob", [35, P], BF16,
                                       isOutput=False)
    out = nc.declare_dram_parameter("out", [P, 48], F32, isOutput=True)
    if debug:
        dbg_st = nc.declare_dram_parameter("dbg_st", [P, 8], F32,
                                           isOutput=True)
        dbg_g = nc.declare_dram_parameter("dbg_g", [P, 2], F32, isOutput=True)
        dbg_qp = nc.declare_dram_parameter("dbg_qp", [3, NSL], F32,
                                           isOutput=True)

    with TileContext(nc) as tc:
        with (
            tc.tile_pool(name="singles", bufs=1) as singles,
            tc.tile_pool(name="sb", bufs=2) as sb,
            tc.tile_pool(name="stile", bufs=2) as stile,
            tc.tile_pool(name="psum_bc", bufs=2, space="PSUM") as psbc,
            tc.tile_pool(name="psum_q", bufs=1, space="PSUM") as psq,
            tc.tile_pool(name="psum_w", bufs=2, space="PSUM") as psw,
            tc.tile_pool(name="psum_as", bufs=1, space="PSUM") as psas,
        ):
            # ---- input DMAs (five queues; grid-gating loads first) ----
            imgstack_sb = singles.tile([35, N], BF16)
            bfblob_sb = singles.tile([35, P], BF16)
            fblob_sb = singles.tile([P, FB_W], F32)
            qimg_r = singles.tile([3, NSL], F32R)
            nc.sync.dma_start(out=imgstack_sb[0:15, 0:NSL],
                              in_=imgstack[:, 0:NSL])
            nc.scalar.dma_start(out=imgstack_sb[32:35, 0:NSL],
                                in_=imghi3[:, 0:NSL])
            nc.gpsimd.dma_start(out=bfblob_sb[:], in_=bfblob[:])
            nc.sync.dma_start(out=imgstack_sb[0:15, NSL:N],
                              in_=imgstack[:, NSL:N])
            nc.scalar.dma_start(out=imgstack_sb[32:35, NSL:N],
                                in_=imghi3[:, NSL:N])
            nc.gpsimd.dma_start(out=fblob_sb[:], in_=fblob[:])
            nc.scalar.dma_start(out=qimg_r[:], in_=qimg[:])

            imgv = fblob_sb[:, FB_IMG:FB_IMG + 96].rearrange(
                "p (c t) -> p c t", c=3)
            texp = fblob_sb[:, FB_TEXP:FB_TEXP + 1]

            ones128 = singles.tile([P, 1], F32)
            nc.vector.memset(ones128[:], 1.0)
            ones1 = singles.tile([1, P], F32)
            nc.vector.memset(ones1[:], 1.0)
            # anchor the activation-table load before any DMA-gated work
            anchor = singles.tile([1, 1], F32)
            nc.scalar.activation(anchor[:], ones1[0:1, 0:1], AF.Exp)

            # f32r copies of tiny matmul operands
            wstk3_r = singles.tile([3, P], F32R)
            nc.vector.tensor_copy(wstk3_r[:], fblob_sb[0:3, FB_WSTK:FB_WSTK + P])
            wqT_r = singles.tile([3, 3], F32R)
            nc.vector.tensor_copy(wqT_r[:], fblob_sb[0:3, FB_WQT:FB_WQT + 3])
            lum_r = singles.tile([3, 3], F32R)
            nc.vector.tensor_copy(lum_r[:], fblob_sb[0:3, FB_LUM:FB_LUM + 3])

            # ---- luma stats pass 1 (column layout) ----
            Lc = sb.tile([P, 32], F32, tag="Lc")
            nc.vector.tensor_scalar(Lc[:], imgv[:, 0, :], LUMW[0], None,
                                    op0=ALU.mult)
            nc.vector.scalar_tensor_tensor(Lc[:], in0=imgv[:, 1, :],
                                           scalar=LUMW[1], in1=Lc[:],
                                           op0=ALU.mult, op1=ALU.add)
            nc.vector.scalar_tensor_tensor(Lc[:], in0=imgv[:, 2, :],
                                           scalar=LUMW[2], in1=Lc[:],
                                           op0=ALU.mult, op1=ALU.add)
            red2 = sb.tile([P, 2], F32, tag="red2")
            nc.vector.tensor_reduce(red2[:, 0:1], Lc[:], axis=AX.X, op=ALU.add)
            l2junk = sb.tile([P, 32], F32, tag="l2junk")
            nc.vector.scalar_tensor_tensor(l2junk[:], in0=Lc[:], scalar=1.0,
                                           in1=Lc[:], op0=ALU.bypass,
                                           op1=ALU.mult,
                                           accum_out=red2[:, 1:2])

            # ---- grid chunks: 8 x 512 keys ----
            dparts = sb.tile([P, 8], F32, tag="dparts")
            nparts = sb.tile([P, 8], F32, tag="nparts")

            def grid_chunk(ch):
                off = ch * 512
                kb_ps = psbc.tile([P, 512], F32, tag="bc")
                nc.tensor.matmul(kb_ps[:], lhsT=bfblob_sb[0:15, :],
                                 rhs=imgstack_sb[0:15, off:off + 512],
                                 start=True, stop=True)
                s_t = stile.tile([P, 512], BF16, tag="s")
                nc.scalar.activation(s_t[:], kb_ps[:], AF.Exp,
                                     accum_out=dparts[:, ch:ch + 1])
                vb_ps = psbc.tile([P, 512], F32, tag="bc")
                nc.tensor.matmul(vb_ps[:], lhsT=bfblob_sb[32:35, :],
                                 rhs=imgstack_sb[32:35, off:off + 512],
                                 start=True, stop=True)
                junk = stile.tile([P, 512], BF16, tag="junk")
                nc.vector.scalar_tensor_tensor(
                    junk[:], in0=s_t[:], scalar=1.0, in1=vb_ps[:],
                    op0=ALU.bypass, op1=ALU.mult,
                    accum_out=nparts[:, ch:ch + 1])

            grid_chunk(0)
            grid_chunk(1)

            # ---- stats partition-reduce round 1 ----
            pstat = psas.tile([P, 104], F32)  # att cols 0:96, stats 96:104
            nc.tensor.matmul(pstat[0:1, 96:98], lhsT=ones128[:],
                             rhs=red2[:], start=True, stop=True)
            sums12 = sb.tile([1, 2], F32, tag="sums12")
            nc.vector.tensor_copy(sums12[:], pstat[0:1, 96:98])
            nc.tensor.matmul(pstat[:, 98:100], lhsT=ones1[:], rhs=sums12[:],
                             start=True, stop=True)
            mu_bc = singles.tile([P, 1], F32)
            nc.vector.tensor_scalar(mu_bc[:], pstat[:, 98:99], 1.0 / N, None,
                                    op0=ALU.mult)
            mneg = singles.tile([P, 1], F32)
            nc.vector.tensor_scalar(mneg[:], mu_bc[:], -1.0, None,
                                    op0=ALU.mult)

            grid_chunk(2)
            grid_chunk(3)

            # ---- stats pass 2: sum |L - mu| then var/rinv ----
            adj = sb.tile([P, 32], F32, tag="adj")
            adsum = sb.tile([P, 1], F32, tag="adsum")
            nc.vector.tensor_scalar(adj[:], Lc[:], mu_bc[:, 0:1], None,
                                    op0=ALU.subtract)
            nc.vector.tensor_reduce(adsum[:], adj[:], axis=AX.X, op=ALU.add,
                                    apply_absolute_value=True)
            nc.tensor.matmul(pstat[0:1, 100:101], lhsT=ones128[:],
                             rhs=adsum[:], start=True, stop=True)
            # var = (sdL2 - sA^2/N)/(N-1); sdL2 = sL2 - sL^2/N
            sc = sb.tile([1, 8], F32, tag="sc")
            nc.vector.tensor_copy(sc[:, 0:1], pstat[0:1, 100:101])  # sA
            nc.vector.tensor_tensor(sc[:, 1:2], sums12[:, 0:1],
                                    sums12[:, 0:1], op=ALU.mult)  # sL^2
            nc.vector.scalar_tensor_tensor(sc[:, 2:3], in0=sc[:, 1:2],
                                           scalar=-1.0 / N,
                                           in1=sums12[:, 1:2],
                                           op0=ALU.mult, op1=ALU.add)  # sdL2
            nc.vector.tensor_tensor(sc[:, 3:4], sc[:, 0:1], sc[:, 0:1],
                                    op=ALU.mult)  # sA^2
            nc.vector.scalar_tensor_tensor(sc[:, 4:5], in0=sc[:, 3:4],
                                           scalar=-1.0 / N, in1=sc[:, 2:3],
                                           op0=ALU.mult, op1=ALU.add)
            nc.vector.tensor_scalar(sc[:, 5:6], sc[:, 4:5], 1.0 / (N - 1),
                                    None, op0=ALU.mult)  # var
            # std = sqrt(var) via 2 Newton steps from a range-tuned seed
            sq = sb.tile([1, 4], F32, tag="sq")
            nc.vector.memset(sq[:, 0:1], 0.105)
            for _ in range(2):
                nc.vector.reciprocal(sq[:, 1:2], sq[:, 0:1])
                nc.vector.tensor_tensor(sq[:, 2:3], sc[:, 5:6], sq[:, 1:2],
                                        op=ALU.mult)
                nc.vector.tensor_scalar(sq[:, 3:4], sq[:, 0:1], 0.5, None,
                                        op0=ALU.mult)
                nc.vector.scalar_tensor_tensor(sq[:, 0:1], in0=sq[:, 2:3],
                                               scalar=0.5, in1=sq[:, 3:4],
                                               op0=ALU.mult, op1=ALU.add)
            nc.vector.tensor_scalar(sc[:, 6:7], sq[:, 0:1], 1e-6, None,
                                    op0=ALU.add)
            nc.vector.reciprocal(sc[:, 7:8], sc[:, 6:7])
            rh1 = sb.tile([1, 1], F32, tag="rh1")
            nc.vector.tensor_scalar(rh1[:], sc[:, 7:8], 0.5, None,
                                    op0=ALU.mult)  # 0.5/(std+eps)
            nc.tensor.matmul(pstat[:, 101:102], lhsT=ones1[:], rhs=rh1[:],
                             start=True, stop=True)
            rh_bc = singles.tile([P, 1], F32)
            nc.vector.tensor_copy(rh_bc[:], pstat[:, 101:102])
            tnb = singles.tile([P, 1], F32)
            nc.vector.tensor_tensor(tnb[:], mneg[:], rh_bc[:], op=ALU.mult)
            if debug:
                st_dbg = singles.tile([P, 8], F32)
                nc.vector.tensor_copy(st_dbg[:, 0:1], mu_bc[:])
                nc.vector.tensor_copy(st_dbg[:, 1:2], rh_bc[:])
                nc.vector.memset(st_dbg[:, 2:8], 0.0)
                nc.sync.dma_start(out=dbg_st[:], in_=st_dbg[:])

            for _ch in range(4, 8):
                grid_chunk(_ch)

            # ---- q side: per half, Lq/q matmuls + gate + W + att ----
            w_sb = singles.tile([P, NSL], BF16)
            qp_r = singles.tile([3, NSL], F32R)
            if debug:
                qp_dbg = singles.tile([3, NSL], F32)

            def q_half(half):
                for c2 in range(2):
                    off = half * 1024 + c2 * 512
                    Lq_ps = psq.tile([3, 512], F32, tag="lq")
                    q_ps = psq.tile([3, 512], F32, tag="q")
                    nc.tensor.matmul(Lq_ps[:], lhsT=lum_r[:],
                                     rhs=qimg_r[:, off:off + 512],
                                     start=True, stop=True)
                    nc.tensor.matmul(q_ps[:], lhsT=wqT_r[:],
                                     rhs=qimg_r[:, off:off + 512],
                                     start=True, stop=True)
                    th = sb.tile([3, 512], F32, tag="th")
                    nc.scalar.activation(th[:], Lq_ps[:], AF.Tanh,
                                         scale=rh_bc[0:3, 0:1],
                                         bias=tnb[0:3, 0:1])
                    ath = sb.tile([3, 512], F32, tag="ath")
                    nc.vector.scalar_tensor_tensor(ath[:], in0=th[:],
                                                   scalar=-1.0, in1=th[:],
                                                   op0=ALU.mult, op1=ALU.max)
                    g1 = sb.tile([3, 512], F32, tag="g1")
                    nc.gpsimd.tensor_scalar(g1[:], ath[:], 0.5, 1.5,
                                            op0=ALU.mult, op1=ALU.add)
                    nc.vector.tensor_tensor(qp_r[:, off:off + 512],
                                            q_ps[:], g1[:], op=ALU.mult)
                    if debug:
                        nc.vector.tensor_copy(qp_dbg[:, off:off + 512],
                                              qp_r[:, off:off + 512])
                    wp = psw.tile([P, 512], F32, tag="w")
                    nc.tensor.matmul(wp[:], lhsT=wstk3_r[:],
                                     rhs=qp_r[:, off:off + 512],
                                     start=True, stop=True)
                    nc.scalar.activation(w_sb[:, off:off + 512], wp[:],
                                         AF.Exp, bias=texp[:, 0:1])

            q_half(0)
            q_half(1)
            if debug:
                nc.sync.dma_start(out=dbg_qp[:], in_=qp_dbg[:])

            # ---- g2: per-head grid sums in bf16, block-diagonal ----
            g2f = sb.tile([P, 2], F32, tag="g2f")
            nc.vector.tensor_reduce(g2f[:, 0:1], nparts[:], axis=AX.X,
                                    op=ALU.add)
            nc.vector.tensor_reduce(g2f[:, 1:2], dparts[:], axis=AX.X,
                                    op=ALU.add)
            if debug:
                nc.sync.dma_start(out=dbg_g[:], in_=g2f[:])
            g2z = singles.tile([P, 3, 2], BF16)
            nc.vector.memset(g2z[:], 0.0)
            for h in range(3):
                nc.vector.tensor_copy(g2z[32 * h:32 * h + 32, h, :],
                                      g2f[32 * h:32 * h + 32, :])

            # ---- att (16 col-major matmuls) + tail, pipelined in two
            # qt-groups so divide/mix/clip overlap the second att batch ----
            att_ps = pstat[:, 0:96].rearrange("p (q h d) -> p q h d",
                                              q=NQT, h=3)
            outv = out.rearrange("p (c q) -> p c q", c=3)
            out_sb = singles.tile([P, 3, NQT], F32)
            for g in range(2):
                qsl = slice(g * 8, g * 8 + 8)
                for qt in range(g * 8, g * 8 + 8):
                    nc.tensor.matmul(
                        att_ps[:, qt, :, :],
                        lhsT=w_sb[:, qt * P:(qt + 1) * P],
                        rhs=g2z[:].rearrange("p h two -> p (h two)"),
                        start=True, stop=True)
                rden = sb.tile([P, 8, 3], F32, tag="rden")
                nc.vector.reciprocal(rden[:], att_ps[:, qsl, :, 1])
                attc = sb.tile([P, 8, 3], F32, tag="attc")
                nc.vector.tensor_tensor(attc[:], att_ps[:, qsl, :, 0],
                                        rden[:], op=ALU.mult)
                for cch in range(3):
                    mix = sb.tile([P, 8], F32, tag="mix")
                    nc.vector.tensor_scalar(
                        mix[:], attc[:, :, 0],
                        fblob_sb[:, FB_WO + 3 * cch:FB_WO + 3 * cch + 1],
                        None, op0=ALU.mult)
                    for h in (1, 2):
                        nc.vector.scalar_tensor_tensor(
                            mix[:], in0=attc[:, :, h],
                            scalar=fblob_sb[:, FB_WO + 3 * cch + h:
                                            FB_WO + 3 * cch + h + 1],
                            in1=mix[:], op0=ALU.mult, op1=ALU.add)
                    nc.vector.tensor_tensor(mix[:], mix[:],
                                            imgv[:, cch, qsl], op=ALU.add)
                    nc.vector.tensor_scalar(out_sb[:, cch, qsl], mix[:],
                                            0.0, 1.0, op0=ALU.max,
                                            op1=ALU.min)
                nc.sync.dma_start(out=outv[:, :, qsl],
                                  in_=out_sb[:, :, qsl])

    nc.finalize()
    return nc


_NC_CACHE = {}


def _get_nc(debug=False):
    key = ("dbg" if debug else "nc")
    if key not in _NC_CACHE:
        _NC_CACHE[key] = build_nc(debug)
    return _NC_CACHE[key]


def make_in_maps(rgb, wq, wk, wv, wo):
    import ml_dtypes
    BF = ml_dtypes.bfloat16
    x4 = np.ascontiguousarray(rgb.reshape(4, 3, N)).astype(np.float32)

    # per-head quadrature grids in partition blocks [32h, 32h+32)
    pairs = [(0, 0), (1, 1), (2, 2), (0, 1), (0, 2), (1, 2)]
    wkt = np.zeros((3, P), np.float32)
    wk2 = np.zeros((6, P), np.float32)
    wvb = np.zeros((3, P), np.float32)
    wstk3 = np.zeros((3, P), np.float32)
    texp = np.full((P, 1), -100.0, np.float32)
    for h in range(3):
        A = 2.0 * float(np.abs(wq[h]).sum()) + 0.5
        sig = 0.0
        for _ in range(4):
            Rh = A + 6.0 * sig
            hg = 2.0 * Rh / (T - 1)
            sig = hg
        t = (-Rh + np.arange(T) * hg).astype(np.float32)
        sl = slice(32 * h, 32 * h + T)
        wkt[:, sl] = wk[h][:, None] * t[None, :]
        for pi, (c, cp) in enumerate(pairs):
            coef = (-0.5 * sig * sig * wk[h][c] * wk[h][cp]
                    * (1.0 if c == cp else 2.0))
            wk2[pi, sl] = coef
        wvb[:, sl] = wv[h][:, None]
        wstk3[h, sl] = t / (sig * sig)
        texp[sl, 0] = -t * t / (2.0 * sig * sig)
    wkthi = wkt.astype(BF)
    wktlo = (wkt - wkthi.astype(np.float32)).astype(BF)
    bfblob = np.zeros((35, P), BF)
    bfblob[0:15] = np.concatenate([wkthi, wkthi, wktlo, wk2.astype(BF)],
                                  axis=0)
    bfblob[32:35] = wvb.astype(BF)

    lumw = np.array(LUMW, np.float32)
    fb_const = np.zeros((P, FB_W - FB_TEXP), np.float32)
    fb_const[:, 0:1] = texp
    fb_const[:, FB_WO - FB_TEXP:FB_WO - FB_TEXP + 9] = np.tile(
        wo.reshape(1, 9), (P, 1))
    fb_const[0:3, FB_WSTK - FB_TEXP:FB_WSTK - FB_TEXP + P] = wstk3
    fb_const[0:3, FB_WQT - FB_TEXP:FB_WQT - FB_TEXP + 3] = \
        np.ascontiguousarray(wq.T)
    fb_const[0:3, FB_LUM - FB_TEXP:FB_LUM - FB_TEXP + 3] = np.tile(
        lumw[:, None], (1, 3))

    in_maps = []
    for j in range(8):
        b, half = j // 2, j % 2
        x = x4[b]
        xhi = x.astype(BF)
        xlo = (x - xhi.astype(np.float32)).astype(BF)
        x2 = np.stack([x[c] * x[cp] for (c, cp) in pairs]).astype(BF)
        imgstack = np.ascontiguousarray(
            np.concatenate([xhi, xlo, xhi, x2], axis=0))
        # column layout [p, c, ct]; this core's 16 query col-tiles first so
        # the residual/clip tail reads cols [0:16)
        tiles = x.reshape(3, 32, P)
        order = list(range(16 * half, 16 * half + 16)) + \
            list(range(16 * (1 - half), 16 * (1 - half) + 16))
        imgcol = np.ascontiguousarray(
            tiles[:, order, :].transpose(2, 0, 1).reshape(P, 96))
        fblob = np.concatenate([imgcol, fb_const], axis=1)
        qs = np.ascontiguousarray(x[:, half * NSL:(half + 1) * NSL])
        in_maps.append({
            "fblob": np.ascontiguousarray(fblob),
            "imgstack": imgstack,
            "imghi3": np.ascontiguousarray(xhi),
            "qimg": qs,
            "bfblob": bfblob,
        })
    return in_maps


def run(rgb, wq, wk, wv, wo, trace=False, debug=False):
    nc = _get_nc(debug)
    in_maps = make_in_maps(rgb, wq, wk, wv, wo)
    res = run_bass_kernel_spmd(nc, in_maps, core_ids=list(range(8)),
                               trace=trace)
    y = np.zeros((4, 3, N), dtype=np.float32)
    for j in range(8):
        b, half = j // 2, j % 2
        sl = slice(half * NSL, (half + 1) * NSL)
        o = res.results[j]["out"]
        y[b][:, sl] = o.reshape(P, 3, NQT).transpose(1, 2, 0).reshape(3, NSL)
    return y.reshape(4, 3, 64, 64), res


def kernel(**inputs):
    args = {k: np.asarray(inputs[k], dtype=np.float32)
            for k in ("rgb", "wq", "wk", "wv", "wo")}
    y, _ = run(args["rgb"], args["wq"], args["wk"], args["wv"], args["wo"])
    return y
